# revision 38
# baseline (speedup 1.0000x reference)
"""Trainium2 Bass kernel for the Box-diamond histogram-binning module.

Reference math (B=4096, D=4096, BIN_T=8, BIN1=4, P=512):
  xr[b,p,l] = x[b, (p//4)*32 + l*4 + (p%4)]           (p = u*4+j, u in [0,128))
  W1[p,m,l] = sigmoid((l-m)*(m + t2[p] - l))          -> depends only on (d=l-m, p)
  S[b,p,m]  = sum_l ln(1 - xr[b,p,l]*W1[p,m,l])
  y1        = 1/(1-S)            (== -1/(-1+S))
  W2[p,l]   = sigmoid((l-t0)*(t1-l)) * sigmoid((7-t2-l)*l)
  out[b,p]  = 1/(1 - sum_l ln(1 - y1[b,p,l]*W2[p,l]))

Kernel strategy (8 cores, batch-sharded, 512 rows each):
  * partitions = u (128); free dims carry (b, l, j).  x is pre-transposed on
    host to [u, b, c] (c = l*4+j) so DMAs are contiguous per partition.
  * w_d[p] = sigmoid(d*(t2[p]-d)) decays fast in |d|: W1 is banded in
    d = l-m.  Tap d=0 is an ACT pass ln(1 - w_0*x) (per-partition scale
    -w_0[p], bias 1) written straight into S; taps d in {1,-1,2} are ACT
    passes into zero-padded full-width T tiles that the TensorEngine sums
    into PSUM via identity-weight float32r matmuls (1 cyc/row); taps
    d in {-2,3} (w <= 0.018) use ln(1-w*x) ~= -w*x fused into one DVE
    scalar_tensor_tensor op each; taps d=-3 and |d|>=4 (w <= 1.3e-4) are
    dropped.  DVE drains PSUM into S with one add per chunk.
  * Outer stage avoids reciprocal: ln(1 - y1*W2) = ln(1-W2-S) - ln(1-S),
    with per-partition bias 1-W2[p,l]; sums on DVE (sub + reduce).
  * Final 1/(1-T) = exp(-ln(1-T)) on ACT (Ln+Exp pinned to one table set).
  * All sigmoid/W2 prep is done on host (tiny) and shipped as aux tensors.
  * Device output is [u, (j, b)]; host reassembles to [b, p].
  * Cost-model timeline: ~122 us/core (ACT-bound; ACT ~103, DVE ~78,
    PE ~39, DMA ~27).
"""

import numpy as np

import concourse.bass as bass
import concourse.bacc as bacc
import concourse.mybir as mybir
import concourse.tile as tile
from concourse.bass_utils import run_bass_kernel_spmd

F32 = mybir.dt.float32
F32R = mybir.dt.float32r
AF = mybir.ActivationFunctionType

N_CORES = 8
B_FULL = 4096
D_IN = 4096
P = 512
U = 128          # partition dim (p // 4)
J = 4            # p % 4
L = 8            # BIN_T
B_LOC = B_FULL // N_CORES   # 512 batch rows per core
BC = 128                    # inner chunk batch rows
BH = 256                    # "half": outer-stage granularity
N_CHUNK = B_LOC // BC       # 4
N_HALF = B_LOC // BH        # 2

# taps, order matters (d=0 first: it initializes S).  "exact" taps get an
# ACT ln pass; "linear" taps (w_d <= 0.018) use ln(1-w*x) ~= -w*x fused into
# one DVE scalar_tensor_tensor op.
D_EXACT = (0, 1, -1, 2)
D_LIN = (-2, 3)
# issue order: d=0 initializes S, then the cheap DVE linear taps (fill DVE's
# early-chunk idle and release x early), then the ACT taps.
D_ALL = (0, -2, 3, 1, -1, 2)


def _host_aux(t0: np.ndarray, t1: np.ndarray, t2: np.ndarray):
    """Precompute per-p scales/biases on host. Returns (aux1, aux2) f32.

    aux1[u, k*4+j] = -sigmoid(d_k * (t2[p]-d_k)),  p = u*4+j, k indexes D_EXACT
    aux2[u, j*8+l] = 1 - W2[p, l]
    """
    t0 = t0.astype(np.float64)
    t1 = t1.astype(np.float64)
    t2 = t2.astype(np.float64)

    def sig(z):
        return 1.0 / (1.0 + np.exp(-z))

    aux1 = np.empty((U, len(D_ALL) * J), np.float32)
    for k, d in enumerate(D_ALL):
        w = sig(d * (t2 - d))            # [P]
        wm = w.reshape(U, J)             # p = u*4+j
        aux1[:, k * J:(k + 1) * J] = (-wm).astype(np.float32)

    l = np.arange(L, dtype=np.float64)
    w2 = sig((l[None, :] - t0[:, None]) * (t1[:, None] - l[None, :])) \
        * sig((L - 1 - t2[:, None] - l[None, :]) * l[None, :])   # [P, L]
    aux2 = (1.0 - w2).reshape(U, J, L).reshape(U, J * L).astype(np.float32)
    return aux1, aux2


_IDENT = np.eye(U, dtype=np.float32)


def _win(d):
    """valid l-range [lo, hi) for tap d; output m = l - d in [lo-d, hi-d)."""
    lo = max(0, d)
    hi = min(L, L + d)
    return lo, hi - lo


_NC_CACHE = None


def _pin_act_table_set():
    """Make the table-load pass resolve Ln and Exp to the single set that
    contains both (natural_log_exp_and_others), avoiding per-switch ~1.3us
    table reloads between the inner (Ln) and final (Exp) stages."""
    from concourse.bacc import get_activation_tables
    tabs = get_activation_tables("gen3")
    both = tabs.get("natural_log_exp_and_others")
    if not both or AF.Ln not in both or AF.Exp not in both:
        return
    for name, fns in tabs.items():
        if name == "natural_log_exp_and_others":
            continue
        fns.discard(AF.Ln)
        fns.discard(AF.Exp)



def _build_program():
    global _NC_CACHE
    if _NC_CACHE is not None:
        return _NC_CACHE

    _pin_act_table_set()
    nc = bacc.Bacc("TRN2", target_bir_lowering=False, debug=False,
                   num_devices=N_CORES)
    # x pre-transposed on host: [u, b*32 + c] with c = l*4 + j
    x_d = nc.dram_tensor("xr", [U, B_LOC * 32], F32, kind="ExternalInput")
    a1_d = nc.dram_tensor("aux1", [U, len(D_ALL) * J], F32,
                          kind="ExternalInput")
    a2_d = nc.dram_tensor("aux2", [U, J * L], F32, kind="ExternalInput")
    id_d = nc.dram_tensor("ident", [U, U], F32, kind="ExternalInput")
    # device-layout output: [u, j*B_LOC + b]
    o_d = nc.dram_tensor("outr", [U, J * B_LOC], F32, kind="ExternalOutput")
    ov = o_d.ap().rearrange("u (j b) -> u j b", j=J)

    with tile.TileContext(nc) as tc:
        with (
            tc.tile_pool(name="aux", bufs=1) as auxp,
            tc.tile_pool(name="x", bufs=2) as xp,
            tc.tile_pool(name="t", bufs=1) as tp,
            tc.tile_pool(name="s", bufs=1) as sp,
            tc.tile_pool(name="outer", bufs=2) as op_,
            tc.tile_pool(name="outer1", bufs=1) as o1p,
            tc.tile_pool(name="ps", bufs=1, space="PSUM") as pp,
        ):
            a1 = auxp.tile([U, len(D_ALL) * J], F32)
            nc.sync.dma_start(out=a1[:], in_=a1_d.ap())
            a2 = auxp.tile([U, J * L], F32)
            nc.sync.dma_start(out=a2[:], in_=a2_d.ap())
            idt = auxp.tile([U, U], F32R)
            nc.gpsimd.dma_start(out=idt[:], in_=id_d.ap())

            # S[u, (b, j, m)] for all 512 local batch rows, accumulated
            # in place chunk by chunk; outer stage runs once at the end so
            # its 32 per-(j,l) bias instructions amortize over b=512.
            S = sp.tile([U, B_LOC * J * L], F32)
            Sv = S[:].rearrange("u (b j m) -> u b j m", b=B_LOC, j=J, m=L)

            # one persistent full-width T tile per PE tap; pad columns are
            # zeroed once here and never written again (ACT only writes the
            # valid window, PE reads the full tile).
            D_PE = tuple(d for d in D_EXACT if d != 0)
            D_MM = (1, -1, 2)  # taps summed on the TensorEngine (f32r)
            Ttiles = {}
            for d in D_PE:
                lo, win = _win(d)
                mlo = lo - d
                T = tp.tile([U, BC * 32], F32R, tag=f"T{d}")
                Tv = T[:].rearrange("u (b j m) -> u b j m", b=BC, j=J, m=L)
                if mlo > 0:
                    nc.gpsimd.memset(Tv[:, :, :, 0:mlo].bitcast(F32), 0.0)
                if mlo + win < L:
                    nc.gpsimd.memset(Tv[:, :, :, mlo + win:L].bitcast(F32), 0.0)
                Ttiles[d] = (T, Tv)

            for c in range(N_CHUNK):
                gb = c * BC        # local batch offset
                xt = xp.tile([U, BC * 32], F32)
                hb = BC // 2
                if c == 0:
                    # split the first chunk's load so ACT starts sooner
                    qb = BC // 4
                    for q in range(4):
                        nc.gpsimd.dma_start(
                            out=xt[:, q * qb * 32:(q + 1) * qb * 32],
                            in_=x_d.ap()[:, q * qb * 32:(q + 1) * qb * 32])
                else:
                    nc.gpsimd.dma_start(
                        out=xt[:], in_=x_d.ap()[:, gb * 32:(gb + BC) * 32])
                xv = xt[:].rearrange("u (b l j) -> u b l j", b=BC, l=L, j=J)
                Sc = Sv[:, gb:gb + BC]

                # d=0 initializes this chunk of S directly
                k0 = D_ALL.index(0)
                qb = BC // 4
                bsplits = (tuple((q * qb, (q + 1) * qb) for q in range(4))
                           if c == 0 else ((0, BC),))
                for b0, b1 in bsplits:
                    for j in range(J):
                        nc.scalar.activation(
                            Sc[:, b0:b1, j, :], xv[:, b0:b1, :, j],
                            AF.Ln, bias=1.0,
                            scale=a1[:, k0 * J + j:k0 * J + j + 1],
                        )
                # linear taps fused into S on DVE (fill DVE's early idle)
                for d in D_LIN:
                    k = D_ALL.index(d)
                    lo, win = _win(d)
                    mlo = lo - d
                    for j in range(J):
                        acc = Sc[:, :, j, mlo:mlo + win]
                        nc.vector.scalar_tensor_tensor(
                            acc, xv[:, :, lo:lo + win, j],
                            a1[:, k * J + j:k * J + j + 1], acc,
                            op0=mybir.AluOpType.mult,
                            op1=mybir.AluOpType.add,
                        )
                # remaining exact taps: ACT -> full-width T tiles, summed
                # into PSUM by PE identity-matmuls (f32r, 1 cyc/row; pads
                # are zero so full-width accumulation is safe)
                PS = pp.tile([U, BC * 32], F32)
                n_mm = len(D_MM)
                for ki, d in enumerate(D_PE):
                    k = D_ALL.index(d)
                    lo, win = _win(d)
                    mlo = lo - d
                    T, Tv = Ttiles[d]
                    for j in range(J):
                        nc.scalar.activation(
                            Tv[:, :, j, mlo:mlo + win],
                            xv[:, :, lo:lo + win, j], AF.Ln,
                            bias=1.0, scale=a1[:, k * J + j:k * J + j + 1],
                        )
                    if d in D_MM:
                        mi = D_MM.index(d)
                        for nb in range(BC * 32 // 512):
                            cs = slice(nb * 512, (nb + 1) * 512)
                            nc.tensor.matmul(
                                PS[:, cs], idt[:], T[:, cs],
                                start=(mi == 0), stop=(mi == n_mm - 1),
                            )
                    else:
                        Sf = S[:, gb * 32:(gb + BC) * 32]
                        nc.vector.tensor_add(Sf, Sf, T[:])
                # drain: S += PS (DVE, PSUM-src tensor_tensor)
                if c == N_CHUNK - 1:
                    # per-j so each j's S completes independently and the
                    # outer stage can start early
                    PSv = PS[:].rearrange("u (b j m) -> u b j m",
                                          b=BC, j=J, m=L)
                    for j in range(J):
                        nc.vector.tensor_add(Sc[:, :, j, :], Sc[:, :, j, :],
                                             PSv[:, :, j, :])
                else:
                    Sf = S[:, gb * 32:(gb + BC) * 32]
                    nc.vector.tensor_add(Sf, Sf, PS[:])

            # ---- outer stage, once over all 512 rows ----
            # R[u, (j, b)] = sum_l [ln(1-W2-S) - ln(1-S)]
            R = o1p.tile([U, J * B_LOC], F32)
            Rv = R[:].rearrange("u (j b) -> u j b", j=J)
            for j in range(J):
                TA = op_.tile([U, L * B_LOC], F32)
                TAv = TA[:].rearrange("u (l b) -> u l b", l=L)
                for li in range(L):
                    nc.scalar.activation(
                        TAv[:, li, :], Sv[:, :, j, li], AF.Ln,
                        bias=a2[:, j * L + li:j * L + li + 1], scale=-1.0,
                    )
                TB = o1p.tile([U, L * B_LOC], F32)
                TBv = TB[:].rearrange("u (l b) -> u l b", l=L)
                TAr = TA[:].rearrange("u (l b) -> u b l", l=L)
                V = o1p.tile([U, B_LOC], F32)
                O = op_.tile([U, B_LOC], F32)
                # last j: finer splits so its serial tail chain pipelines
                nsp = 4 if j == J - 1 else 2
                HB = B_LOC // nsp
                for b0 in range(0, B_LOC, HB):
                    bs = slice(b0, b0 + HB)
                    # halved so the j=3 tail chain pipelines
                    nc.scalar.activation(
                        TBv[:, :, bs],
                        Sv[:, bs, j, :].transpose([0, 2, 1]), AF.Ln,
                        bias=1.0, scale=-1.0,
                    )
                    nc.vector.tensor_sub(TAv[:, :, bs], TAv[:, :, bs],
                                         TBv[:, :, bs])
                    nc.vector.tensor_reduce(
                        Rv[:, j, bs], TAr[:, bs],
                        axis=mybir.AxisListType.X, op=mybir.AluOpType.add,
                    )
                    # out = exp(-ln(1 - R)) for this j, then DMA out
                    nc.scalar.activation(V[:, bs], Rv[:, j, bs], AF.Ln,
                                         bias=1.0, scale=-1.0)
                    nc.scalar.activation(O[:, bs], V[:, bs], AF.Exp,
                                         bias=0.0, scale=-1.0)
                    nc.sync.dma_start(out=ov[:, j, bs], in_=O[:, bs])

    nc.finalize()
    _NC_CACHE = nc
    return nc


def run(x, t0, t1, t2, trace=False, **kw):
    import os
    if not trace:
        # the axon client in this container has no NTFF profiling hook;
        # make sure an inherited BASS_TRACE=1 cannot push us onto that path
        os.environ["BASS_NEVER_TRACE"] = "1"
    x = np.asarray(x, dtype=np.float32)
    aux1, aux2 = _host_aux(np.asarray(t0), np.asarray(t1), np.asarray(t2))
    # host pre-transpose: [B, 4096] -> per core [u, b_loc, c] contiguous
    xt = x.reshape(B_FULL, U, 32).transpose(1, 0, 2)   # [u, B, 32] (view)
    nc = _build_program()
    in_maps = []
    for c in range(N_CORES):
        xc = np.ascontiguousarray(
            xt[:, c * B_LOC:(c + 1) * B_LOC, :]).reshape(U, B_LOC * 32)
        in_maps.append({"xr": xc, "aux1": aux1, "aux2": aux2,
                        "ident": _IDENT})
    res = run_bass_kernel_spmd(nc, in_maps, core_ids=list(range(N_CORES)),
                               trace=trace, **kw)
    # device layout [u, (j, b_loc)] -> [b, p] with p = u*4+j
    out = np.empty((B_FULL, P), np.float32)
    for c in range(N_CORES):
        oc = res.results[c]["outr"].reshape(U, J, B_LOC)
        out[c * B_LOC:(c + 1) * B_LOC] = oc.transpose(2, 0, 1).reshape(B_LOC, P)
    return out, res


def kernel(x, t0, t1, t2):
    out, _ = run(x, t0, t1, t2)
    return out


# revision 40
# speedup vs baseline: 1.0128x; 1.0128x over previous
"""Trainium2 Bass kernel for the Box-diamond histogram-binning module.

Reference math (B=4096, D=4096, BIN_T=8, BIN1=4, P=512):
  xr[b,p,l] = x[b, (p//4)*32 + l*4 + (p%4)]           (p = u*4+j, u in [0,128))
  W1[p,m,l] = sigmoid((l-m)*(m + t2[p] - l))          -> depends only on (d=l-m, p)
  S[b,p,m]  = sum_l ln(1 - xr[b,p,l]*W1[p,m,l])
  y1        = 1/(1-S)            (== -1/(-1+S))
  W2[p,l]   = sigmoid((l-t0)*(t1-l)) * sigmoid((7-t2-l)*l)
  out[b,p]  = 1/(1 - sum_l ln(1 - y1[b,p,l]*W2[p,l]))

Kernel strategy (8 cores, batch-sharded, 512 rows each):
  * partitions = u (128); free dims carry (b, l, j).  x is pre-transposed on
    host to [u, b, c] (c = l*4+j) so DMAs are contiguous per partition.
  * w_d[p] = sigmoid(d*(t2[p]-d)) decays fast in |d|: W1 is banded in
    d = l-m.  Tap d=0 is an ACT pass ln(1 - w_0*x) (per-partition scale
    -w_0[p], bias 1) written straight into S; taps d in {1,-1,2} are ACT
    passes into zero-padded full-width T tiles that the TensorEngine sums
    into PSUM via identity-weight float32r matmuls (1 cyc/row); taps
    d in {-2,3} (w <= 0.018) use ln(1-w*x) ~= -w*x fused into one DVE
    scalar_tensor_tensor op each; taps d=-3 and |d|>=4 (w <= 1.3e-4) are
    dropped.  DVE drains PSUM into S with one add per chunk.
  * Outer stage avoids reciprocal: ln(1 - y1*W2) = ln(1-W2-S) - ln(1-S),
    with per-partition bias 1-W2[p,l]; sums on DVE (sub + reduce).
  * Final 1/(1-T) = exp(-ln(1-T)) on ACT (Ln+Exp pinned to one table set).
  * All sigmoid/W2 prep is done on host (tiny) and shipped as aux tensors.
  * Device output is [u, (j, b)]; host reassembles to [b, p].
  * Cost-model timeline: ~122 us/core (ACT-bound; ACT ~103, DVE ~78,
    PE ~39, DMA ~27).
"""

import numpy as np

import concourse.bass as bass
import concourse.bacc as bacc
import concourse.mybir as mybir
import concourse.tile as tile
from concourse.bass_utils import run_bass_kernel_spmd

F32 = mybir.dt.float32
F32R = mybir.dt.float32r
AF = mybir.ActivationFunctionType

N_CORES = 8
B_FULL = 4096
D_IN = 4096
P = 512
U = 128          # partition dim (p // 4)
J = 4            # p % 4
L = 8            # BIN_T
B_LOC = B_FULL // N_CORES   # 512 batch rows per core
BC = 128                    # inner chunk batch rows
BH = 256                    # "half": outer-stage granularity
N_CHUNK = B_LOC // BC       # 4
N_HALF = B_LOC // BH        # 2

# taps, order matters (d=0 first: it initializes S).  "exact" taps get an
# ACT ln pass; "linear" taps (w_d <= 0.018) use ln(1-w*x) ~= -w*x fused into
# one DVE scalar_tensor_tensor op.
D_EXACT = (0, 1, -1, 2)
D_LIN = (-2, 3)
# issue order: d=0 initializes S, then the cheap DVE linear taps (fill DVE's
# early-chunk idle and release x early), then the ACT taps.
D_ALL = (0, -2, 3, 1, -1, 2)


def _host_aux(t0: np.ndarray, t1: np.ndarray, t2: np.ndarray):
    """Precompute per-p scales/biases on host. Returns (aux1, aux2) f32.

    aux1[u, k*4+j] = -sigmoid(d_k * (t2[p]-d_k)),  p = u*4+j, k indexes D_EXACT
    aux2[u, j*8+l] = -W2[p, l]
    """
    t0 = t0.astype(np.float64)
    t1 = t1.astype(np.float64)
    t2 = t2.astype(np.float64)

    def sig(z):
        return 1.0 / (1.0 + np.exp(-z))

    aux1 = np.empty((U, len(D_ALL) * J), np.float32)
    for k, d in enumerate(D_ALL):
        w = sig(d * (t2 - d))            # [P]
        wm = w.reshape(U, J)             # p = u*4+j
        aux1[:, k * J:(k + 1) * J] = (-wm).astype(np.float32)

    l = np.arange(L, dtype=np.float64)
    w2 = sig((l[None, :] - t0[:, None]) * (t1[:, None] - l[None, :])) \
        * sig((L - 1 - t2[:, None] - l[None, :]) * l[None, :])   # [P, L]
    aux2 = (-w2).reshape(U, J, L).reshape(U, J * L).astype(np.float32)
    return aux1, aux2


_IDENT = np.eye(U, dtype=np.float32)


def _win(d):
    """valid l-range [lo, hi) for tap d; output m = l - d in [lo-d, hi-d)."""
    lo = max(0, d)
    hi = min(L, L + d)
    return lo, hi - lo


_NC_CACHE = None


def _pin_act_table_set():
    """Make the table-load pass resolve Ln and Exp to the single set that
    contains both (natural_log_exp_and_others), avoiding per-switch ~1.3us
    table reloads between the inner (Ln) and final (Exp) stages."""
    from concourse.bacc import get_activation_tables
    tabs = get_activation_tables("gen3")
    both = tabs.get("natural_log_exp_and_others")
    if not both or AF.Ln not in both or AF.Exp not in both:
        return
    for name, fns in tabs.items():
        if name == "natural_log_exp_and_others":
            continue
        fns.discard(AF.Ln)
        fns.discard(AF.Exp)



def _build_program():
    global _NC_CACHE
    if _NC_CACHE is not None:
        return _NC_CACHE

    _pin_act_table_set()
    nc = bacc.Bacc("TRN2", target_bir_lowering=False, debug=False,
                   num_devices=N_CORES)
    # x pre-transposed on host: [u, b*32 + c] with c = l*4 + j
    x_d = nc.dram_tensor("xr", [U, B_LOC * 32], F32, kind="ExternalInput")
    a1_d = nc.dram_tensor("aux1", [U, len(D_ALL) * J], F32,
                          kind="ExternalInput")
    a2_d = nc.dram_tensor("aux2", [U, J * L], F32, kind="ExternalInput")
    id_d = nc.dram_tensor("ident", [U, U], F32, kind="ExternalInput")
    # device-layout output: [u, j*B_LOC + b]
    o_d = nc.dram_tensor("outr", [U, J * B_LOC], F32, kind="ExternalOutput")
    ov = o_d.ap().rearrange("u (j b) -> u j b", j=J)

    with tile.TileContext(nc) as tc:
        with (
            tc.tile_pool(name="aux", bufs=1) as auxp,
            tc.tile_pool(name="x", bufs=2) as xp,
            tc.tile_pool(name="t", bufs=1) as tp,
            tc.tile_pool(name="s", bufs=1) as sp,
            tc.tile_pool(name="outer", bufs=2) as op_,
            tc.tile_pool(name="outer1", bufs=1) as o1p,
            tc.tile_pool(name="ps", bufs=1, space="PSUM") as pp,
        ):
            a1 = auxp.tile([U, len(D_ALL) * J], F32)
            nc.sync.dma_start(out=a1[:], in_=a1_d.ap())
            a2 = auxp.tile([U, J * L], F32)
            nc.sync.dma_start(out=a2[:], in_=a2_d.ap())
            idt = auxp.tile([U, U], F32R)
            nc.gpsimd.dma_start(out=idt[:], in_=id_d.ap())

            # S[u, (b, j, m)] for all 512 local batch rows, accumulated
            # in place chunk by chunk; outer stage runs once at the end so
            # its 32 per-(j,l) bias instructions amortize over b=512.
            S = sp.tile([U, B_LOC * J * L], F32)
            Sv = S[:].rearrange("u (b j m) -> u b j m", b=B_LOC, j=J, m=L)

            # one persistent full-width T tile per PE tap; pad columns are
            # zeroed once here and never written again (ACT only writes the
            # valid window, PE reads the full tile).
            D_PE = tuple(d for d in D_EXACT if d != 0)
            D_MM = (1, -1, 2)  # taps summed on the TensorEngine (f32r)
            Ttiles = {}
            for d in D_PE:
                lo, win = _win(d)
                mlo = lo - d
                T = tp.tile([U, BC * 32], F32R, tag=f"T{d}")
                Tv = T[:].rearrange("u (b j m) -> u b j m", b=BC, j=J, m=L)
                if mlo > 0:
                    nc.gpsimd.memset(Tv[:, :, :, 0:mlo].bitcast(F32), 0.0)
                if mlo + win < L:
                    nc.gpsimd.memset(Tv[:, :, :, mlo + win:L].bitcast(F32), 0.0)
                Ttiles[d] = (T, Tv)

            for c in range(N_CHUNK):
                gb = c * BC        # local batch offset
                xt = xp.tile([U, BC * 32], F32)
                hb = BC // 2
                if c == 0:
                    # split the first chunk's load so ACT starts sooner
                    qb = BC // 4
                    for q in range(4):
                        nc.gpsimd.dma_start(
                            out=xt[:, q * qb * 32:(q + 1) * qb * 32],
                            in_=x_d.ap()[:, q * qb * 32:(q + 1) * qb * 32])
                else:
                    nc.gpsimd.dma_start(
                        out=xt[:], in_=x_d.ap()[:, gb * 32:(gb + BC) * 32])
                xv = xt[:].rearrange("u (b l j) -> u b l j", b=BC, l=L, j=J)
                Sc = Sv[:, gb:gb + BC]

                # d=0 initializes this chunk of S directly
                k0 = D_ALL.index(0)
                qb = BC // 4
                bsplits = (tuple((q * qb, (q + 1) * qb) for q in range(4))
                           if c == 0 else ((0, BC),))
                for b0, b1 in bsplits:
                    for j in range(J):
                        nc.scalar.activation(
                            Sc[:, b0:b1, j, :], xv[:, b0:b1, :, j],
                            AF.Ln, bias=1.0,
                            scale=a1[:, k0 * J + j:k0 * J + j + 1],
                        )
                # linear taps fused into S on DVE (fill DVE's early idle)
                for d in D_LIN:
                    k = D_ALL.index(d)
                    lo, win = _win(d)
                    mlo = lo - d
                    for j in range(J):
                        acc = Sc[:, :, j, mlo:mlo + win]
                        nc.vector.scalar_tensor_tensor(
                            acc, xv[:, :, lo:lo + win, j],
                            a1[:, k * J + j:k * J + j + 1], acc,
                            op0=mybir.AluOpType.mult,
                            op1=mybir.AluOpType.add,
                        )
                # remaining exact taps: ACT -> full-width T tiles, summed
                # into PSUM by PE identity-matmuls (f32r, 1 cyc/row; pads
                # are zero so full-width accumulation is safe)
                PS = pp.tile([U, BC * 32], F32)
                n_mm = len(D_MM)
                for ki, d in enumerate(D_PE):
                    k = D_ALL.index(d)
                    lo, win = _win(d)
                    mlo = lo - d
                    T, Tv = Ttiles[d]
                    for j in range(J):
                        nc.scalar.activation(
                            Tv[:, :, j, mlo:mlo + win],
                            xv[:, :, lo:lo + win, j], AF.Ln,
                            bias=1.0, scale=a1[:, k * J + j:k * J + j + 1],
                        )
                    if d in D_MM:
                        mi = D_MM.index(d)
                        for nb in range(BC * 32 // 512):
                            cs = slice(nb * 512, (nb + 1) * 512)
                            nc.tensor.matmul(
                                PS[:, cs], idt[:], T[:, cs],
                                start=(mi == 0), stop=(mi == n_mm - 1),
                            )
                    else:
                        Sf = S[:, gb * 32:(gb + BC) * 32]
                        nc.vector.tensor_add(Sf, Sf, T[:])
                # drain: S += PS (DVE, PSUM-src tensor_tensor), then
                # convert in place to W = S - 1 (= -(1-S) = -Q)
                if c == N_CHUNK - 1:
                    # per-j so each j's S completes independently and the
                    # outer stage can start early
                    PSv = PS[:].rearrange("u (b j m) -> u b j m",
                                          b=BC, j=J, m=L)
                    for j in range(J):
                        nc.vector.tensor_add(Sc[:, :, j, :], Sc[:, :, j, :],
                                             PSv[:, :, j, :])
                        nc.vector.tensor_scalar_sub(Sc[:, :, j, :],
                                                    Sc[:, :, j, :], 1.0)
                else:
                    Sf = S[:, gb * 32:(gb + BC) * 32]
                    nc.vector.tensor_add(Sf, Sf, PS[:])
                    nc.vector.tensor_scalar_sub(Sf, Sf, 1.0)

            # ---- outer stage, once over all 512 rows ----
            # S now holds W = S-1 = -Q.  Per (b,p):
            #   RA = sum_l ln(Q_l - W2_l)   via ACT scale=-1, bias=-W2
            #   PB = prod_l W_l = prod_l Q_l  (8 factors, signs cancel)
            #   out = 1/(1-T) = exp(-ln(1 + ln PB - RA))
            R = o1p.tile([U, J * B_LOC], F32)
            Rv = R[:].rearrange("u (j b) -> u j b", j=J)
            for j in range(J):
                TA = op_.tile([U, L * B_LOC], F32)
                TAv = TA[:].rearrange("u (l b) -> u l b", l=L)
                for li in range(L):
                    nc.scalar.activation(
                        TAv[:, li, :], Sv[:, :, j, li], AF.Ln,
                        bias=a2[:, j * L + li:j * L + li + 1], scale=-1.0,
                    )
                TAr = TA[:].rearrange("u (l b) -> u b l", l=L)
                Wj = Sv[:, :, j, :].rearrange("u b (l2 two) -> u b l2 two",
                                              two=2)
                T1 = o1p.tile([U, B_LOC * 4], F32)
                T1v = T1[:].rearrange("u (b k) -> u b k", k=4)
                T1p = T1[:].rearrange("u (b k) -> u b k", k=4)\
                    .rearrange("u b (k2 two) -> u b k2 two", two=2)
                T2 = o1p.tile([U, B_LOC * 2], F32)
                T2v = T2[:].rearrange("u (b k) -> u b k", k=2)
                PB = o1p.tile([U, B_LOC], F32)
                V1 = o1p.tile([U, B_LOC], F32)
                V2 = o1p.tile([U, B_LOC], F32)
                O = op_.tile([U, B_LOC], F32)
                # last j: finer splits so its serial tail chain pipelines
                nsp = 4 if j == J - 1 else 2
                HB = B_LOC // nsp
                for b0 in range(0, B_LOC, HB):
                    bs = slice(b0, b0 + HB)
                    nc.vector.tensor_reduce(
                        Rv[:, j, bs], TAr[:, bs],
                        axis=mybir.AxisListType.X, op=mybir.AluOpType.add,
                    )
                    # product tree over l: 8 -> 4 -> 2 -> 1
                    nc.vector.tensor_mul(T1v[:, bs, :], Wj[:, bs, :, 0],
                                         Wj[:, bs, :, 1])
                    nc.vector.tensor_mul(T2v[:, bs, :], T1p[:, bs, :, 0],
                                         T1p[:, bs, :, 1])
                    nc.vector.tensor_mul(PB[:, bs], T2v[:, bs, 0],
                                         T2v[:, bs, 1])
                    # V2 = ln(1 + lnPB - RA); then out = exp(-V2)
                    nc.scalar.activation(V1[:, bs], PB[:, bs], AF.Ln,
                                         bias=0.0, scale=1.0)
                    nc.vector.tensor_sub(V1[:, bs], V1[:, bs], Rv[:, j, bs])
                    nc.scalar.activation(V2[:, bs], V1[:, bs], AF.Ln,
                                         bias=1.0, scale=1.0)
                    nc.scalar.activation(O[:, bs], V2[:, bs], AF.Exp,
                                         bias=0.0, scale=-1.0)
                    nc.sync.dma_start(out=ov[:, j, bs], in_=O[:, bs])

    nc.finalize()
    _NC_CACHE = nc
    return nc


def run(x, t0, t1, t2, trace=False, **kw):
    import os
    if not trace:
        # the axon client in this container has no NTFF profiling hook;
        # make sure an inherited BASS_TRACE=1 cannot push us onto that path
        os.environ["BASS_NEVER_TRACE"] = "1"
    x = np.asarray(x, dtype=np.float32)
    aux1, aux2 = _host_aux(np.asarray(t0), np.asarray(t1), np.asarray(t2))
    # host pre-transpose: [B, 4096] -> per core [u, b_loc, c] contiguous
    xt = x.reshape(B_FULL, U, 32).transpose(1, 0, 2)   # [u, B, 32] (view)
    nc = _build_program()
    in_maps = []
    for c in range(N_CORES):
        xc = np.ascontiguousarray(
            xt[:, c * B_LOC:(c + 1) * B_LOC, :]).reshape(U, B_LOC * 32)
        in_maps.append({"xr": xc, "aux1": aux1, "aux2": aux2,
                        "ident": _IDENT})
    res = run_bass_kernel_spmd(nc, in_maps, core_ids=list(range(N_CORES)),
                               trace=trace, **kw)
    # device layout [u, (j, b_loc)] -> [b, p] with p = u*4+j
    out = np.empty((B_FULL, P), np.float32)
    for c in range(N_CORES):
        oc = res.results[c]["outr"].reshape(U, J, B_LOC)
        out[c * B_LOC:(c + 1) * B_LOC] = oc.transpose(2, 0, 1).reshape(B_LOC, P)
    return out, res


def kernel(x, t0, t1, t2):
    out, _ = run(x, t0, t1, t2)
    return out


# revision 43
# speedup vs baseline: 1.0363x; 1.0232x over previous
"""Trainium2 Bass kernel for the Box-diamond histogram-binning module.

Reference math (B=4096, D=4096, BIN_T=8, BIN1=4, P=512):
  xr[b,p,l] = x[b, (p//4)*32 + l*4 + (p%4)]           (p = u*4+j, u in [0,128))
  W1[p,m,l] = sigmoid((l-m)*(m + t2[p] - l))          -> depends only on (d=l-m, p)
  S[b,p,m]  = sum_l ln(1 - xr[b,p,l]*W1[p,m,l])
  y1        = 1/(1-S)            (== -1/(-1+S))
  W2[p,l]   = sigmoid((l-t0)*(t1-l)) * sigmoid((7-t2-l)*l)
  out[b,p]  = 1/(1 - sum_l ln(1 - y1[b,p,l]*W2[p,l]))

Kernel strategy (8 cores, batch-sharded, 512 rows each):
  * partitions = u (128); free dims carry (b, l, j).  x is pre-transposed on
    host to [u, b, c] (c = l*4+j) so DMAs are contiguous per partition.
  * w_d[p] = sigmoid(d*(t2[p]-d)) decays fast in |d|: W1 is banded in
    d = l-m.  Tap d=0 is an ACT pass ln(1 - w_0*x) (per-partition scale
    -w_0[p], bias 1) written straight into S; taps d in {1,-1,2} are ACT
    passes into zero-padded full-width T tiles that the TensorEngine sums
    into PSUM via identity-weight float32r matmuls (1 cyc/row); taps
    d in {-2,3} (w <= 0.018) use ln(1-w*x) ~= -w*x fused into one DVE
    scalar_tensor_tensor op each; taps d=-3 and |d|>=4 (w <= 1.3e-4) are
    dropped.  DVE drains PSUM into S with one add per chunk.
  * Outer stage avoids reciprocal: T = sum_l ln(1-W2-S_l) - ln(prod_l (1-S_l)).
    After each chunk S is converted in place to W = S-1; the A-pass is ACT
    ln(-W - W2) with per-partition bias -W2[p,l]; the product of the eight
    W_l (= prod (1-S_l), signs cancel) is a 3-level DVE multiply tree, so
    the whole B-term costs one small ACT ln instead of a full ln pass.
  * Final 1/(1-T) = exp(-ln(1 + lnPB - RA)) on ACT (Ln+Exp one table set).
  * All sigmoid/W2 prep is done on host (tiny) and shipped as aux tensors.
  * Device output is [u, (j, b)]; host reassembles to [b, p].
  * Cost-model timeline: ~120 us/core (ACT ~93, DVE ~89, PE ~39, DMA ~27).
"""

import numpy as np

import concourse.bass as bass
import concourse.bacc as bacc
import concourse.mybir as mybir
import concourse.tile as tile
from concourse.bass_utils import run_bass_kernel_spmd

F32 = mybir.dt.float32
F32R = mybir.dt.float32r
AF = mybir.ActivationFunctionType

N_CORES = 8
B_FULL = 4096
D_IN = 4096
P = 512
U = 128          # partition dim (p // 4)
J = 4            # p % 4
L = 8            # BIN_T
B_LOC = B_FULL // N_CORES   # 512 batch rows per core
BC = 128                    # inner chunk batch rows
BH = 256                    # "half": outer-stage granularity
N_CHUNK = B_LOC // BC       # 4
N_HALF = B_LOC // BH        # 2

# taps, order matters (d=0 first: it initializes S).  "exact" taps get an
# ACT ln pass; "linear" taps (w_d <= 0.018) use ln(1-w*x) ~= -w*x fused into
# one DVE scalar_tensor_tensor op.
D_EXACT = (0, 1, -1, 2)
D_LIN = (-2, 3)
# issue order: d=0 initializes S, then the cheap DVE linear taps (fill DVE's
# early-chunk idle and release x early), then the ACT taps.
D_ALL = (0, -2, 3, 1, -1, 2)


def _host_aux(t0: np.ndarray, t1: np.ndarray, t2: np.ndarray):
    """Precompute per-p scales/biases on host. Returns (aux1, aux2) f32.

    aux1[u, k*4+j] = -sigmoid(d_k * (t2[p]-d_k)),  p = u*4+j, k indexes D_EXACT
    aux2[u, j*8+l] = -W2[p, l]
    """
    t0 = t0.astype(np.float64)
    t1 = t1.astype(np.float64)
    t2 = t2.astype(np.float64)

    def sig(z):
        return 1.0 / (1.0 + np.exp(-z))

    aux1 = np.empty((U, len(D_ALL) * J), np.float32)
    for k, d in enumerate(D_ALL):
        w = sig(d * (t2 - d))            # [P]
        wm = w.reshape(U, J)             # p = u*4+j
        aux1[:, k * J:(k + 1) * J] = (-wm).astype(np.float32)

    l = np.arange(L, dtype=np.float64)
    w2 = sig((l[None, :] - t0[:, None]) * (t1[:, None] - l[None, :])) \
        * sig((L - 1 - t2[:, None] - l[None, :]) * l[None, :])   # [P, L]
    aux2 = (-w2).reshape(U, J, L).reshape(U, J * L).astype(np.float32)
    return aux1, aux2


_IDENT = np.eye(U, dtype=np.float32)


def _win(d):
    """valid l-range [lo, hi) for tap d; output m = l - d in [lo-d, hi-d)."""
    lo = max(0, d)
    hi = min(L, L + d)
    return lo, hi - lo


_NC_CACHE = None


def _pin_act_table_set():
    """Make the table-load pass resolve Ln and Exp to the single set that
    contains both (natural_log_exp_and_others), avoiding per-switch ~1.3us
    table reloads between the inner (Ln) and final (Exp) stages."""
    from concourse.bacc import get_activation_tables
    tabs = get_activation_tables("gen3")
    both = tabs.get("natural_log_exp_and_others")
    if not both or AF.Ln not in both or AF.Exp not in both:
        return
    for name, fns in tabs.items():
        if name == "natural_log_exp_and_others":
            continue
        fns.discard(AF.Ln)
        fns.discard(AF.Exp)



def _build_program():
    global _NC_CACHE
    if _NC_CACHE is not None:
        return _NC_CACHE

    _pin_act_table_set()
    nc = bacc.Bacc("TRN2", target_bir_lowering=False, debug=False,
                   num_devices=N_CORES)
    # x pre-transposed on host: [u, b*32 + c] with c = l*4 + j
    x_d = nc.dram_tensor("xr", [U, B_LOC * 32], F32, kind="ExternalInput")
    a1_d = nc.dram_tensor("aux1", [U, len(D_ALL) * J], F32,
                          kind="ExternalInput")
    a2_d = nc.dram_tensor("aux2", [U, J * L], F32, kind="ExternalInput")
    id_d = nc.dram_tensor("ident", [U, U], F32, kind="ExternalInput")
    # device-layout output: [u, j*B_LOC + b]
    o_d = nc.dram_tensor("outr", [U, J * B_LOC], F32, kind="ExternalOutput")
    ov = o_d.ap().rearrange("u (j b) -> u j b", j=J)

    with tile.TileContext(nc) as tc:
        with (
            tc.tile_pool(name="aux", bufs=1) as auxp,
            tc.tile_pool(name="x", bufs=2) as xp,
            tc.tile_pool(name="t", bufs=1) as tp,
            tc.tile_pool(name="s", bufs=1) as sp,
            tc.tile_pool(name="outer", bufs=2) as op_,
            tc.tile_pool(name="outer1", bufs=1) as o1p,
            tc.tile_pool(name="ps", bufs=1, space="PSUM") as pp,
        ):
            a1 = auxp.tile([U, len(D_ALL) * J], F32)
            nc.sync.dma_start(out=a1[:], in_=a1_d.ap())
            a2 = auxp.tile([U, J * L], F32)
            nc.sync.dma_start(out=a2[:], in_=a2_d.ap())
            idt = auxp.tile([U, U], F32R)
            nc.gpsimd.dma_start(out=idt[:], in_=id_d.ap())

            # S[u, (b, j, m)] for all 512 local batch rows, accumulated
            # in place chunk by chunk; outer stage runs once at the end so
            # its 32 per-(j,l) bias instructions amortize over b=512.
            S = sp.tile([U, B_LOC * J * L], F32)
            Sv = S[:].rearrange("u (b j m) -> u b j m", b=B_LOC, j=J, m=L)

            # one persistent full-width T tile per PE tap; pad columns are
            # zeroed once here and never written again (ACT only writes the
            # valid window, PE reads the full tile).
            D_PE = tuple(d for d in D_EXACT if d != 0)
            D_MM = (1, -1, 2)  # taps summed on the TensorEngine (f32r)
            Ttiles = {}
            for d in D_PE:
                lo, win = _win(d)
                mlo = lo - d
                T = tp.tile([U, BC * 32], F32R, tag=f"T{d}")
                Tv = T[:].rearrange("u (b j m) -> u b j m", b=BC, j=J, m=L)
                if mlo > 0:
                    nc.gpsimd.memset(Tv[:, :, :, 0:mlo].bitcast(F32), 0.0)
                if mlo + win < L:
                    nc.gpsimd.memset(Tv[:, :, :, mlo + win:L].bitcast(F32), 0.0)
                Ttiles[d] = (T, Tv)

            for c in range(N_CHUNK):
                gb = c * BC        # local batch offset
                xt = xp.tile([U, BC * 32], F32)
                hb = BC // 2
                if c == 0:
                    # split the first chunk's load so ACT starts sooner
                    qb = BC // 4
                    for q in range(4):
                        nc.gpsimd.dma_start(
                            out=xt[:, q * qb * 32:(q + 1) * qb * 32],
                            in_=x_d.ap()[:, q * qb * 32:(q + 1) * qb * 32])
                else:
                    nc.gpsimd.dma_start(
                        out=xt[:], in_=x_d.ap()[:, gb * 32:(gb + BC) * 32])
                xv = xt[:].rearrange("u (b l j) -> u b l j", b=BC, l=L, j=J)
                Sc = Sv[:, gb:gb + BC]

                # d=0 initializes this chunk of S directly
                k0 = D_ALL.index(0)
                qb = BC // 4
                bsplits = (tuple((q * qb, (q + 1) * qb) for q in range(4))
                           if c == 0 else ((0, BC),))
                for b0, b1 in bsplits:
                    for j in range(J):
                        nc.scalar.activation(
                            Sc[:, b0:b1, j, :], xv[:, b0:b1, :, j],
                            AF.Ln, bias=1.0,
                            scale=a1[:, k0 * J + j:k0 * J + j + 1],
                        )
                # linear taps fused into S on DVE (fill DVE's early idle)
                for d in D_LIN:
                    k = D_ALL.index(d)
                    lo, win = _win(d)
                    mlo = lo - d
                    for j in range(J):
                        acc = Sc[:, :, j, mlo:mlo + win]
                        nc.vector.scalar_tensor_tensor(
                            acc, xv[:, :, lo:lo + win, j],
                            a1[:, k * J + j:k * J + j + 1], acc,
                            op0=mybir.AluOpType.mult,
                            op1=mybir.AluOpType.add,
                        )
                # remaining exact taps: ACT -> full-width T tiles, summed
                # into PSUM by PE identity-matmuls (f32r, 1 cyc/row; pads
                # are zero so full-width accumulation is safe)
                PS = pp.tile([U, BC * 32], F32)
                n_mm = len(D_MM)
                for ki, d in enumerate(D_PE):
                    k = D_ALL.index(d)
                    lo, win = _win(d)
                    mlo = lo - d
                    T, Tv = Ttiles[d]
                    for j in range(J):
                        nc.scalar.activation(
                            Tv[:, :, j, mlo:mlo + win],
                            xv[:, :, lo:lo + win, j], AF.Ln,
                            bias=1.0, scale=a1[:, k * J + j:k * J + j + 1],
                        )
                    if d in D_MM:
                        mi = D_MM.index(d)
                        for nb in range(BC * 32 // 512):
                            cs = slice(nb * 512, (nb + 1) * 512)
                            nc.tensor.matmul(
                                PS[:, cs], idt[:], T[:, cs],
                                start=(mi == 0), stop=(mi == n_mm - 1),
                            )
                    else:
                        Sf = S[:, gb * 32:(gb + BC) * 32]
                        nc.vector.tensor_add(Sf, Sf, T[:])
                # drain: S += PS (DVE, PSUM-src tensor_tensor), then
                # convert in place to W = S - 1 (= -(1-S) = -Q)
                if c == N_CHUNK - 1:
                    # per-j so each j's S completes independently and the
                    # outer stage can start early
                    PSv = PS[:].rearrange("u (b j m) -> u b j m",
                                          b=BC, j=J, m=L)
                    for j in range(J):
                        nc.vector.scalar_tensor_tensor(
                            Sc[:, :, j, :], Sc[:, :, j, :], 1.0,
                            PSv[:, :, j, :],
                            op0=mybir.AluOpType.subtract,
                            op1=mybir.AluOpType.add)
                else:
                    Sf = S[:, gb * 32:(gb + BC) * 32]
                    nc.vector.scalar_tensor_tensor(
                        Sf, Sf, 1.0, PS[:],
                        op0=mybir.AluOpType.subtract,
                        op1=mybir.AluOpType.add)

            # ---- outer stage, once over all 512 rows ----
            # S now holds W = S-1 = -Q.  Per (b,p):
            #   RA = sum_l ln(Q_l - W2_l)   via ACT scale=-1, bias=-W2
            #   PB = prod_l W_l = prod_l Q_l  (8 factors, signs cancel)
            #   out = 1/(1-T) = exp(-ln(1 + ln PB - RA))
            R = o1p.tile([U, J * B_LOC], F32)
            Rv = R[:].rearrange("u (j b) -> u j b", j=J)
            for j in range(J):
                TA = op_.tile([U, L * B_LOC], F32)
                TAv = TA[:].rearrange("u (l b) -> u l b", l=L)
                for li in range(L):
                    nc.scalar.activation(
                        TAv[:, li, :], Sv[:, :, j, li], AF.Ln,
                        bias=a2[:, j * L + li:j * L + li + 1], scale=-1.0,
                    )
                TAr = TA[:].rearrange("u (l b) -> u b l", l=L)
                Wj = Sv[:, :, j, :].rearrange("u b (l2 two) -> u b l2 two",
                                              two=2)
                T1 = o1p.tile([U, B_LOC * 4], F32)
                T1v = T1[:].rearrange("u (b k) -> u b k", k=4)
                T1p = T1[:].rearrange("u (b k) -> u b k", k=4)\
                    .rearrange("u b (k2 two) -> u b k2 two", two=2)
                T2 = o1p.tile([U, B_LOC * 2], F32)
                T2v = T2[:].rearrange("u (b k) -> u b k", k=2)
                PB = o1p.tile([U, B_LOC], F32)
                V1 = o1p.tile([U, B_LOC], F32)
                V2 = o1p.tile([U, B_LOC], F32)
                O = op_.tile([U, B_LOC], F32)
                # last j: finer splits so its serial tail chain pipelines
                nsp = 4 if j == J - 1 else 2
                HB = B_LOC // nsp
                for b0 in range(0, B_LOC, HB):
                    bs = slice(b0, b0 + HB)
                    # product tree over l first: depends only on S(j), so
                    # DVE streams without waiting for the ACT A-pass
                    nc.vector.tensor_mul(T1v[:, bs, :], Wj[:, bs, :, 0],
                                         Wj[:, bs, :, 1])
                    nc.vector.tensor_mul(T2v[:, bs, :], T1p[:, bs, :, 0],
                                         T1p[:, bs, :, 1])
                    nc.vector.tensor_mul(PB[:, bs], T2v[:, bs, 0],
                                         T2v[:, bs, 1])
                    nc.scalar.activation(V1[:, bs], PB[:, bs], AF.Ln,
                                         bias=0.0, scale=1.0)
                    nc.vector.tensor_reduce(
                        Rv[:, j, bs], TAr[:, bs],
                        axis=mybir.AxisListType.X, op=mybir.AluOpType.add,
                    )
                    # V2 = ln(1 + lnPB - RA); then out = exp(-V2)
                    nc.vector.tensor_sub(V1[:, bs], V1[:, bs], Rv[:, j, bs])
                    nc.scalar.activation(V2[:, bs], V1[:, bs], AF.Ln,
                                         bias=1.0, scale=1.0)
                    nc.scalar.activation(O[:, bs], V2[:, bs], AF.Exp,
                                         bias=0.0, scale=-1.0)
                    nc.sync.dma_start(out=ov[:, j, bs], in_=O[:, bs])

    nc.finalize()
    _NC_CACHE = nc
    return nc


def run(x, t0, t1, t2, trace=False, **kw):
    import os
    if not trace:
        # the axon client in this container has no NTFF profiling hook;
        # make sure an inherited BASS_TRACE=1 cannot push us onto that path
        os.environ["BASS_NEVER_TRACE"] = "1"
    x = np.asarray(x, dtype=np.float32)
    aux1, aux2 = _host_aux(np.asarray(t0), np.asarray(t1), np.asarray(t2))
    # host pre-transpose: [B, 4096] -> per core [u, b_loc, c] contiguous
    xt = x.reshape(B_FULL, U, 32).transpose(1, 0, 2)   # [u, B, 32] (view)
    nc = _build_program()
    in_maps = []
    for c in range(N_CORES):
        xc = np.ascontiguousarray(
            xt[:, c * B_LOC:(c + 1) * B_LOC, :]).reshape(U, B_LOC * 32)
        in_maps.append({"xr": xc, "aux1": aux1, "aux2": aux2,
                        "ident": _IDENT})
    res = run_bass_kernel_spmd(nc, in_maps, core_ids=list(range(N_CORES)),
                               trace=trace, **kw)
    # device layout [u, (j, b_loc)] -> [b, p] with p = u*4+j
    out = np.empty((B_FULL, P), np.float32)
    for c in range(N_CORES):
        oc = res.results[c]["outr"].reshape(U, J, B_LOC)
        out[c * B_LOC:(c + 1) * B_LOC] = oc.transpose(2, 0, 1).reshape(B_LOC, P)
    return out, res


def kernel(x, t0, t1, t2):
    out, _ = run(x, t0, t1, t2)
    return out
